# revision 20
# baseline (speedup 1.0000x reference)
"""Trainium2 Bass kernel for nn_AttentionCT (channel attention / XCA-style).

Reference computation per batch image b:
    y    = depthwise_conv3x3(x_b)                       (192, 128, 128)
    q,k,v = 1x1 conv (qkv_w) on y, split into 8 heads of 24 channels
    q,k  = L2-normalized along the spatial dim (hw = 16384)
    attn = softmax(q @ k^T * temp) per head (24x24); out = attn @ v
    final = proj_w @ out

Key algebraic collapse used here: because the L2 norms and the q@k^T
contraction are both along the SAME spatial axis, everything between the
depthwise conv and the final projection is a function of the 192x192 Gram
matrix G_y = y @ y^T:
    S_full = Wq G_y Wk^T,  qq = diag(Wq G_y Wq^T),  kk = diag(Wk G_y Wk^T)
    logits = S_full / (sqrt(qq) sqrt(kk)^T) * temp   (per-head 24x24 blocks)
    attn   = softmax(logits);  R = blockdiag(attn) @ Wv;  G = proj_w @ R
    final  = G @ y
So the device work is: dwconv (9 diagonal-stationary PE matmuls, fp32r),
a Gram accumulation over 128 transposed column chunks, tiny 192-scale
algebra + softmax, and one fused (192,192) @ (192,16384) output matmul.

Sharding: data-parallel over batch — core i handles x[i]; weights replicated.
"""

import sys

for _p in ("/opt/trn_rl_repo",):
    if _p not in sys.path:
        sys.path.insert(0, _p)

import numpy as np

import concourse.bass as bass
import concourse.bacc as bacc
import concourse.mybir as mybir
import concourse.tile as tile
from concourse.bass_utils import run_bass_kernel_spmd

F32 = mybir.dt.float32
F32R = mybir.dt.float32r
AF = mybir.ActivationFunctionType
ALU = mybir.AluOpType
AX = mybir.AxisListType

C, H, W = 192, 128, 128
NCORES = 8
TAPS = [(dy, dx) for dy in (-1, 0, 1) for dx in (-1, 0, 1)]


def _r(ap):
    return ap.bitcast(F32R)


def build():
    nc = bacc.Bacc(None, target_bir_lowering=False, debug=False)

    x_d = nc.dram_tensor("x", [C, H, W], F32R, kind="ExternalInput")
    dwdiag_d = nc.dram_tensor("dwdiag", [2, 128, 9, 128], F32R, kind="ExternalInput")
    wqt_d = nc.dram_tensor("wqt", [C, C], F32, kind="ExternalInput")
    wkt_d = nc.dram_tensor("wkt", [C, C], F32, kind="ExternalInput")
    wqn_d = nc.dram_tensor("wqn", [C, C], F32, kind="ExternalInput")
    wv_d = nc.dram_tensor("wv", [C, C], F32, kind="ExternalInput")
    projt_d = nc.dram_tensor("projt", [C, C], F32, kind="ExternalInput")
    tcol_d = nc.dram_tensor("tcol", [C, 1], F32, kind="ExternalInput")
    ident_d = nc.dram_tensor("ident", [128, 128], F32, kind="ExternalInput")
    mask_d = nc.dram_tensor("mask", [2, 96, C], F32, kind="ExternalInput")
    out_d = nc.dram_tensor("out", [C, H, W], F32, kind="ExternalOutput")

    with tile.TileContext(nc) as tc:
        with (
            tc.tile_pool(name="weights", bufs=1) as wpool,
            tc.tile_pool(name="xpad", bufs=2) as xpool,
            tc.tile_pool(name="diag", bufs=2) as dpool,
            tc.tile_pool(name="ybuf", bufs=1) as ypool,
            tc.tile_pool(name="ytbuf", bufs=3) as ytpool,
            tc.tile_pool(name="ostage", bufs=3) as opool,
            tc.tile_pool(name="smalls", bufs=1) as spool,
        ):
            # ---- constants / weights ----
            wqt0 = wpool.tile([128, C], F32)
            wqt1 = wpool.tile([64, C], F32)
            nc.sync.dma_start(wqt0[:], wqt_d[0:128, :])
            nc.sync.dma_start(wqt1[:], wqt_d[128:192, :])
            wkt0 = wpool.tile([128, C], F32)
            wkt1 = wpool.tile([64, C], F32)
            nc.sync.dma_start(wkt0[:], wkt_d[0:128, :])
            nc.sync.dma_start(wkt1[:], wkt_d[128:192, :])
            wqn0 = wpool.tile([96, C], F32)
            wqn1 = wpool.tile([96, C], F32)
            nc.sync.dma_start(wqn0[:], wqn_d[0:96, :])
            nc.sync.dma_start(wqn1[:], wqn_d[96:192, :])
            wv0 = wpool.tile([96, C], F32)
            wv1 = wpool.tile([96, C], F32)
            nc.sync.dma_start(wv0[:], wv_d[0:96, :])
            nc.sync.dma_start(wv1[:], wv_d[96:192, :])
            pjt0 = wpool.tile([96, C], F32)
            pjt1 = wpool.tile([96, C], F32)
            nc.sync.dma_start(pjt0[:], projt_d[0:96, :])
            nc.sync.dma_start(pjt1[:], projt_d[96:192, :])
            tc0 = wpool.tile([96, 1], F32)
            tc1 = wpool.tile([96, 1], F32)
            nc.sync.dma_start(tc0[:], tcol_d[0:96, :])
            nc.sync.dma_start(tc1[:], tcol_d[96:192, :])
            ident = wpool.tile([128, 128], F32)
            nc.sync.dma_start(ident[:], ident_d[:])
            mask0 = wpool.tile([96, C], F32)
            mask1 = wpool.tile([96, C], F32)
            nc.sync.dma_start(mask0[:], mask_d[0])
            nc.sync.dma_start(mask1[:], mask_d[1])
            ones128 = wpool.tile([128, 1], F32)
            nc.vector.memset(ones128[:], 1.0)
            ones64 = wpool.tile([64, 1], F32)
            nc.vector.memset(ones64[:], 1.0)

            # ---- y buffers ----
            # y0: channels 0..127 full image; y1: channels 128..191 packed as
            # two row-halves on the partition axis (lanes 0-63 rows 0..63,
            # lanes 64-127 rows 64..127).
            y0 = ypool.tile([128, H, W], F32R)
            y1 = ypool.tile([128, 64, W], F32R)

            # pass-1 PSUM pools (closed before the smalls/final phases so the
            # 8 banks can be re-used)
            _dwps_cm = tc.tile_pool(name="dwps", bufs=2, space=bass.MemorySpace.PSUM)
            dwps = _dwps_cm.__enter__()
            _trps_cm = tc.tile_pool(name="trps", bufs=2, space=bass.MemorySpace.PSUM)
            trps = _trps_cm.__enter__()
            _grps_cm = tc.tile_pool(name="gramps", bufs=1, space=bass.MemorySpace.PSUM)
            grps = _grps_cm.__enter__()

            # ---- depthwise conv: 6 sub-phases over a double-buffered padded
            # x window [128, 34, 130]: buffer row j <-> image row base+j-1 per
            # lane group, cols 1..128 real, cols 0/129 zero pad. Each sub-phase
            # produces 32 output rows (8 chunks of 4).
            dg0 = dpool.tile([128, 9, 128], F32R, tag="dg")
            nc.sync.dma_start(dg0[:], dwdiag_d[0])
            dg1 = dpool.tile([128, 9, 128], F32R, tag="dg")
            nc.sync.dma_start(dg1[:], dwdiag_d[1])

            def dw_subphase(diag_t, fills, y_dst):
                """fills: list of (lane_sl, img_row_lo, img_row_hi, buf_row_lo,
                pad_row or None, chan_lo, chan_hi)."""
                xp = xpool.tile([128, 34, 130], F32R, tag="xpad")
                nc.vector.memset(xp[:, :, 0].bitcast(F32), 0.0)
                nc.vector.memset(xp[:, :, 129].bitcast(F32), 0.0)
                for lane_sl, ilo, ihi, blo, pad_row, clo, chi in fills:
                    if pad_row is not None:
                        nc.vector.memset(xp[lane_sl, pad_row, :].bitcast(F32), 0.0)
                    nc.sync.dma_start(
                        xp[lane_sl, blo : blo + (ihi - ilo), 1:129],
                        x_d[clo:chi, ilo:ihi, :],
                    )
                for ch in range(8):
                    rl = ch * 4
                    ps = dwps.tile([128, 4, 128], F32, tag="dw")
                    for t, (dy, dx) in enumerate(TAPS):
                        rhs = xp[:, rl + dy + 1 : rl + dy + 5, dx + 1 : dx + 129]
                        nc.tensor.matmul(
                            ps[:], diag_t[:, t, :], rhs,
                            start=(t == 0), stop=(t == 8),
                        )
                    nc.scalar.copy(y_dst(rl), ps[:])

            ALL = slice(0, 128)
            LO, HI = slice(0, 64), slice(64, 128)
            # channels 0..127: 4 sub-phases of 32 rows each
            for s in range(4):
                base = 32 * s  # first output image row of this sub-phase
                ilo = max(base - 1, 0)
                ihi = min(base + 33, 128)
                blo = 1 if s == 0 else 0
                pad = 0 if s == 0 else (33 if s == 3 else None)
                dw_subphase(
                    dg0,
                    [(ALL, ilo, ihi, blo, pad, 0, 128)],
                    lambda rl, b=base: y0[:, b + rl : b + rl + 4, :],
                )
            # channels 128..191 packed: lanes 0-63 rows 0..63, lanes 64-127 rows 64..127
            for s in range(2):
                baseA = 32 * s          # halfA output rows
                baseB = 64 + 32 * s     # halfB output rows
                fills = []
                if s == 0:
                    fills.append((LO, 0, 33, 1, 0, 128, 192))
                    fills.append((HI, 63, 97, 0, None, 128, 192))
                else:
                    fills.append((LO, 31, 65, 0, None, 128, 192))
                    fills.append((HI, 95, 128, 0, 33, 128, 192))
                dw_subphase(
                    dg1,
                    fills,
                    lambda rl, bA=baseA: y1[:, bA + rl : bA + rl + 4, :],
                )

            # ---- Gram matrix G_y = y y^T via per-row transposes ----
            gram0 = grps.tile([128, 256], F32)
            gram1 = grps.tile([64, 256], F32)
            for rr in range(H):
                tp = trps.tile([128, 192], F32, tag="tp")
                nc.tensor.transpose(tp[:, 0:128], y0[:, rr, :].bitcast(F32), ident[:])
                if rr < 64:
                    src1 = y1[0:64, rr, :]
                    id64 = ident[0:64, 0:64]
                else:
                    src1 = y1[64:128, rr - 64, :]
                    id64 = ident[64:128, 64:128]
                nc.tensor.transpose(tp[:, 128:192], src1.bitcast(F32), id64)
                yt = ytpool.tile([128, 256], F32R, tag="yt")
                nc.scalar.copy(yt[:, 0:192], tp[:])
                nc.vector.memset(yt[:, 192:256].bitcast(F32), 0.0)
                nc.tensor.matmul(
                    gram0[:], yt[:, 0:128], yt[:],
                    start=(rr == 0), stop=(rr == H - 1),
                )
                nc.tensor.matmul(
                    gram1[:], yt[:, 128:192], yt[:],
                    start=(rr == 0), stop=(rr == H - 1),
                )

            gy0 = spool.tile([128, 192], F32)
            gy1 = spool.tile([64, 192], F32)
            nc.scalar.copy(gy0[:], gram0[:, 0:192])
            nc.scalar.copy(gy1[:], gram1[:, 0:192])

            _grps_cm.__exit__(None, None, None)
            _trps_cm.__exit__(None, None, None)
            _dwps_cm.__exit__(None, None, None)
            _sps_cm = tc.tile_pool(name="sps", bufs=4, space=bass.MemorySpace.PSUM)
            sps = _sps_cm.__enter__()

            # ---- tiny 192-scale algebra (all fp32) ----
            # At = G_y @ Wq^T   (= A^T since G_y is symmetric)
            at_ps0 = sps.tile([128, 192], F32, tag="sm")
            at_ps1 = sps.tile([64, 192], F32, tag="sm")
            nc.tensor.matmul(at_ps0[:], gy0[:, 0:128], wqt0[:], start=True, stop=False)
            nc.tensor.matmul(at_ps0[:], gy1[:, 0:128], wqt1[:], start=False, stop=True)
            nc.tensor.matmul(at_ps1[:], gy0[:, 128:192], wqt0[:], start=True, stop=False)
            nc.tensor.matmul(at_ps1[:], gy1[:, 128:192], wqt1[:], start=False, stop=True)
            at0 = spool.tile([128, 192], F32)
            at1 = spool.tile([64, 192], F32)
            nc.scalar.copy(at0[:], at_ps0[:])
            nc.scalar.copy(at1[:], at_ps1[:])

            # Bt = G_y @ Wk^T
            bt_ps0 = sps.tile([128, 192], F32, tag="sm")
            bt_ps1 = sps.tile([64, 192], F32, tag="sm")
            nc.tensor.matmul(bt_ps0[:], gy0[:, 0:128], wkt0[:], start=True, stop=False)
            nc.tensor.matmul(bt_ps0[:], gy1[:, 0:128], wkt1[:], start=False, stop=True)
            nc.tensor.matmul(bt_ps1[:], gy0[:, 128:192], wkt0[:], start=True, stop=False)
            nc.tensor.matmul(bt_ps1[:], gy1[:, 128:192], wkt1[:], start=False, stop=True)
            bt0 = spool.tile([128, 192], F32)
            bt1 = spool.tile([64, 192], F32)
            nc.scalar.copy(bt0[:], bt_ps0[:])
            nc.scalar.copy(bt1[:], bt_ps1[:])

            # A = Wq @ G_y in 96-row tiles (for per-partition qq accumulation)
            a_ps0 = sps.tile([96, 192], F32, tag="sm")
            a_ps1 = sps.tile([96, 192], F32, tag="sm")
            nc.tensor.matmul(a_ps0[:], wqt0[:, 0:96], gy0[:], start=True, stop=False)
            nc.tensor.matmul(a_ps0[:], wqt1[:, 0:96], gy1[:], start=False, stop=True)
            nc.tensor.matmul(a_ps1[:], wqt0[:, 96:192], gy0[:], start=True, stop=False)
            nc.tensor.matmul(a_ps1[:], wqt1[:, 96:192], gy1[:], start=False, stop=True)
            a0 = spool.tile([96, 192], F32)
            a1 = spool.tile([96, 192], F32)
            nc.scalar.copy(a0[:], a_ps0[:])
            nc.scalar.copy(a1[:], a_ps1[:])

            # qq[c] = sum_j A[c,j] * Wq[c,j]  -> rq = rsqrt(qq) * temp
            junk0 = spool.tile([96, 192], F32, tag="junk")
            junk1 = spool.tile([96, 192], F32, tag="junk")
            qq0 = spool.tile([96, 1], F32)
            qq1 = spool.tile([96, 1], F32)
            nc.vector.scalar_tensor_tensor(
                junk0[:], a0[:], 1.0, wqn0[:], op0=ALU.mult, op1=ALU.mult,
                accum_out=qq0[:],
            )
            nc.vector.scalar_tensor_tensor(
                junk1[:], a1[:], 1.0, wqn1[:], op0=ALU.mult, op1=ALU.mult,
                accum_out=qq1[:],
            )
            rq0 = spool.tile([96, 1], F32)
            rq1 = spool.tile([96, 1], F32)
            nc.scalar.activation(qq0[:], qq0[:], AF.Sqrt)
            nc.scalar.activation(qq1[:], qq1[:], AF.Sqrt)
            nc.vector.reciprocal(rq0[:], qq0[:])
            nc.vector.reciprocal(rq1[:], qq1[:])
            nc.vector.tensor_mul(rq0[:], rq0[:], tc0[:])
            nc.vector.tensor_mul(rq1[:], rq1[:], tc1[:])

            # kk[d] = sum_i Bt[i,d] * Wk^T[i,d] -> rk broadcast row
            pk0 = spool.tile([128, 192], F32)
            pk1 = spool.tile([64, 192], F32)
            nc.vector.tensor_mul(pk0[:], bt0[:], wkt0[:])
            nc.vector.tensor_mul(pk1[:], bt1[:], wkt1[:])
            kk_ps = sps.tile([1, 192], F32, tag="sm")
            nc.tensor.matmul(kk_ps[:], ones128[:], pk0[:], start=True, stop=False)
            nc.tensor.matmul(kk_ps[:], ones64[:], pk1[:], start=False, stop=True)
            rk_row = spool.tile([1, 192], F32)
            nc.scalar.activation(rk_row[:], kk_ps[:], AF.Sqrt)
            nc.vector.reciprocal(rk_row[:], rk_row[:])
            rkb0 = spool.tile([96, 192], F32)
            rkb1 = spool.tile([96, 192], F32)
            nc.gpsimd.partition_broadcast(rkb0[:], rk_row[:])
            nc.gpsimd.partition_broadcast(rkb1[:], rk_row[:])

            # S = A @ Wk^T in 96-row tiles
            s_ps0 = sps.tile([96, 192], F32, tag="sm")
            s_ps1 = sps.tile([96, 192], F32, tag="sm")
            nc.tensor.matmul(s_ps0[:], at0[:, 0:96], wkt0[:], start=True, stop=False)
            nc.tensor.matmul(s_ps0[:], at1[:, 0:96], wkt1[:], start=False, stop=True)
            nc.tensor.matmul(s_ps1[:], at0[:, 96:192], wkt0[:], start=True, stop=False)
            nc.tensor.matmul(s_ps1[:], at1[:, 96:192], wkt1[:], start=False, stop=True)
            s0 = spool.tile([96, 192], F32)
            s1 = spool.tile([96, 192], F32)
            nc.scalar.copy(s0[:], s_ps0[:])
            nc.scalar.copy(s1[:], s_ps1[:])
            nc.vector.tensor_scalar_mul(s0[:], s0[:], rq0[:])
            nc.vector.tensor_mul(s0[:], s0[:], rkb0[:])
            nc.vector.tensor_scalar_mul(s1[:], s1[:], rq1[:])
            nc.vector.tensor_mul(s1[:], s1[:], rkb1[:])

            # Mask off-block logits to -BIG, softmax over the full row, and
            # transpose the resulting block-diagonal attention per 96-group.
            BIG = 1.0e4
            nc.vector.tensor_scalar_add(s0[:], s0[:], BIG)
            nc.vector.tensor_mul(s0[:], s0[:], mask0[:])
            nc.vector.tensor_scalar_add(s0[:], s0[:], -BIG)
            nc.vector.tensor_scalar_add(s1[:], s1[:], BIG)
            nc.vector.tensor_mul(s1[:], s1[:], mask1[:])
            nc.vector.tensor_scalar_add(s1[:], s1[:], -BIG)

            def softmax(sm_t):
                mx = spool.tile([96, 1], F32, tag="mx")
                nc.vector.tensor_reduce(mx[:], sm_t[:], axis=AX.X, op=ALU.max)
                nmx = spool.tile([96, 1], F32, tag="nmx")
                nc.vector.tensor_scalar_mul(nmx[:], mx[:], -1.0)
                nc.scalar.activation(sm_t[:], sm_t[:], AF.Exp, bias=nmx[:], scale=1.0)
                sm = spool.tile([96, 1], F32, tag="smr")
                nc.vector.tensor_reduce(sm[:], sm_t[:], axis=AX.X, op=ALU.add)
                rs = spool.tile([96, 1], F32, tag="rs")
                nc.vector.reciprocal(rs[:], sm[:])
                nc.vector.tensor_scalar_mul(sm_t[:], sm_t[:], rs[:])

            softmax(s0)
            softmax(s1)

            # bdt = attn^T per 96-group via PE transpose (s0 blocks live in
            # cols 0..95, s1 blocks in cols 96..191)
            bd_ps0 = sps.tile([96, 96], F32, tag="sm")
            bd_ps1 = sps.tile([96, 96], F32, tag="sm")
            nc.tensor.transpose(bd_ps0[:], s0[:, 0:96], ident[0:96, 0:96])
            nc.tensor.transpose(bd_ps1[:], s1[:, 96:192], ident[0:96, 0:96])
            bdt0 = spool.tile([96, 96], F32)
            bdt1 = spool.tile([96, 96], F32)
            nc.scalar.copy(bdt0[:], bd_ps0[:])
            nc.scalar.copy(bdt1[:], bd_ps1[:])
            # R = blockdiag(attn) @ Wv, rows grouped 96/96
            r_ps0 = sps.tile([96, 192], F32, tag="sm")
            r_ps1 = sps.tile([96, 192], F32, tag="sm")
            nc.tensor.matmul(r_ps0[:], bdt0[:], wv0[:], start=True, stop=True)
            nc.tensor.matmul(r_ps1[:], bdt1[:], wv1[:], start=True, stop=True)
            rr0 = spool.tile([96, 192], F32)
            rr1 = spool.tile([96, 192], F32)
            nc.scalar.copy(rr0[:], r_ps0[:])
            nc.scalar.copy(rr1[:], r_ps1[:])

            # Gt = R^T @ projT  (so that final = Gt^T @ y = G @ y)
            gt_ps0 = sps.tile([128, 192], F32, tag="sm")
            gt_ps1 = sps.tile([128, 192], F32, tag="sm")
            nc.tensor.matmul(gt_ps0[:], rr0[:, 0:128], pjt0[:], start=True, stop=False)
            nc.tensor.matmul(gt_ps0[:], rr1[:, 0:128], pjt1[:], start=False, stop=True)
            # Gt rows 128..191 are written twice (partition bases 0 and 64) so
            # the final matmul can pair them with y1 slices at either base.
            for pbase in (0, 64):
                nc.tensor.matmul(gt_ps1[pbase : pbase + 64, :], rr0[:, 128:192], pjt0[:], start=True, stop=False)
                nc.tensor.matmul(gt_ps1[pbase : pbase + 64, :], rr1[:, 128:192], pjt1[:], start=False, stop=True)
            gt0 = spool.tile([128, 192], F32R)
            gt1 = spool.tile([128, 192], F32R)
            nc.scalar.copy(gt0[:], gt_ps0[:])
            nc.scalar.copy(gt1[:], gt_ps1[:])

            _sps_cm.__exit__(None, None, None)
            _fps_cm = tc.tile_pool(name="fps", bufs=2, space=bass.MemorySpace.PSUM)
            fps = _fps_cm.__enter__()

            # ---- final = G @ y, streamed in 4-row chunks ----
            for ch in range(32):
                r0 = ch * 4
                if r0 < 64:
                    rhs1 = y1[0:64, r0 : r0 + 4, :]
                    g1a = gt1[0:64, 0:128]
                    g1b = gt1[0:64, 128:192]
                else:
                    rhs1 = y1[64:128, r0 - 64 : r0 - 60, :]
                    g1a = gt1[64:128, 0:128]
                    g1b = gt1[64:128, 128:192]
                f0 = fps.tile([128, 4, 128], F32, tag="f0")
                f1 = fps.tile([64, 4, 128], F32, tag="f1")
                rhs0 = y0[:, r0 : r0 + 4, :]
                nc.tensor.matmul(f0[:], gt0[:, 0:128], rhs0, start=True, stop=False)
                nc.tensor.matmul(f0[:], g1a, rhs1, start=False, stop=True)
                nc.tensor.matmul(f1[:], gt0[:, 128:192], rhs0, start=True, stop=False)
                nc.tensor.matmul(f1[:], g1b, rhs1, start=False, stop=True)
                st0 = opool.tile([128, 4, 128], F32, tag="o0")
                st1 = opool.tile([64, 4, 128], F32, tag="o1")
                nc.vector.tensor_copy(st0[:], f0[:])
                nc.scalar.copy(st1[:], f1[:])
                nc.sync.dma_start(out_d[0:128, r0 : r0 + 4, :], st0[:])
                nc.sync.dma_start(out_d[128:192, r0 : r0 + 4, :], st1[:])
            _fps_cm.__exit__(None, None, None)

    nc.compile()
    return nc


_NC = None
LAST_RESULT = None


def _get_nc():
    global _NC
    if _NC is None:
        _NC = build()
    return _NC


def _head_mask():
    """mask[g, c_local, d]: 1 on the head-diagonal 24x24 block of global row
    c = 96*g + c_local, 0 elsewhere."""
    m = np.zeros((2, 96, C), dtype=np.float32)
    for g in range(2):
        for cl in range(96):
            c = 96 * g + cl
            h = c // 24
            m[g, cl, 24 * h : 24 * h + 24] = 1.0
    return m


def kernel(x, dw_w, qkv_w, proj_w, temperature):
    x = np.ascontiguousarray(np.asarray(x, dtype=np.float32))
    dw = np.asarray(dw_w, dtype=np.float32).reshape(C, 9)
    qkv = np.asarray(qkv_w, dtype=np.float32)
    proj = np.asarray(proj_w, dtype=np.float32)
    temp = np.asarray(temperature, dtype=np.float32).ravel()

    dwdiag = np.zeros((2, 128, 9, 128), dtype=np.float32)
    for t in range(9):
        dwdiag[0, :, t, :] = np.diag(dw[0:128, t])
        w64 = dw[128:192, t]
        dwdiag[1, :, t, :] = np.diag(np.concatenate([w64, w64]))

    wq, wk, wv = qkv[0:C], qkv[C : 2 * C], qkv[2 * C : 3 * C]
    feed = dict(
        dwdiag=dwdiag,
        wqt=np.ascontiguousarray(wq.T),
        wkt=np.ascontiguousarray(wk.T),
        wqn=np.ascontiguousarray(wq),
        wv=np.ascontiguousarray(wv),
        projt=np.ascontiguousarray(proj.T),
        tcol=np.repeat(temp, C // 8).reshape(C, 1).astype(np.float32),
        ident=np.eye(128, dtype=np.float32),
        mask=_head_mask(),
    )
    nc = _get_nc()
    in_maps = [dict(feed, x=x[i]) for i in range(NCORES)]
    res = run_bass_kernel_spmd(nc, in_maps, core_ids=list(range(NCORES)))
    global LAST_RESULT
    LAST_RESULT = res
    return np.stack([m["out"] for m in res.results], axis=0)
